# revision 4
# baseline (speedup 1.0000x reference)
"""AllPairsSimilarity Trainium2 kernel (8 NeuronCores, data-parallel over NQ).

Computes, for each query image q of shape (C=640, HW=1024) against a 5-shot
support set (5, 640, 1024):
    proto = sum_shots(support)            # (C, M)  scale-free (mean == sum here)
    shat  = proto / ||proto||_col         # column-normalized over C
    sim   = qhat.T @ shat                 # (HW, M) cosine similarities
    score = mean_n max_m sim[n, m]
q normalization is applied after the max (1/||q_n|| is constant over m).

Sharding: query images split 16 per core; support replicated.
"""
import os
import sys

for _p in ("/opt/trn_rl_repo",):
    if _p not in sys.path and os.path.isdir(_p):
        sys.path.insert(0, _p)

import numpy as np

import concourse.bass as bass  # noqa: E402
import concourse.tile as tile  # noqa: E402
from concourse import bacc, mybir  # noqa: E402
from concourse import bass_utils  # noqa: E402
from concourse.masks import make_identity  # noqa: E402

F32 = mybir.dt.float32
F32R = mybir.dt.float32r
AX_X = mybir.AxisListType.X
MULT = mybir.AluOpType.mult

NQ_SH = 16   # query images per core
CB = 5       # 640 = 5 x 128 channel blocks
N = 1024     # query patches per image (HW)
M = 1024     # support patches
NCORES = 8


def build_bass():
    nc = bacc.Bacc("TRN2", target_bir_lowering=False, debug=False)
    q_d = nc.declare_dram_parameter("q", [NQ_SH, CB * 128, N], F32, isOutput=False)
    s_d = nc.declare_dram_parameter("s", [5, CB * 128, M], F32, isOutput=False)
    out_d = nc.declare_dram_parameter("out", [1, NQ_SH], F32, isOutput=True)

    with tile.TileContext(nc) as tc:
        with tc.tile_pool(name="shat", bufs=CB) as shat_p, \
             tc.tile_pool(name="const", bufs=1) as const_p, \
             tc.tile_pool(name="norms", bufs=1) as norm_p, \
             tc.tile_pool(name="maxc", bufs=1) as maxc_p:

            ones_f = const_p.tile([128, 1], F32, tag="ones_f")
            nc.vector.memset(ones_f[:], 1.0)
            ones_r = const_p.tile([128, 1], F32R, tag="ones_r")
            nc.sync.dma_start(ones_r[:], ones_f[:].bitcast(F32R))
            ident16 = const_p.tile([16, 16], F32, tag="ident")
            make_identity(nc, ident16[:])

            norms2_all = norm_p.tile([NQ_SH, N], F32, tag="n2")
            # column layout: [:, nb*16 + img]
            maxcols = maxc_p.tile([128, 128], F32, tag="mc")

            # ---------------- prologue: support prototype ----------------
            shat = []
            with tc.tile_pool(name="sload", bufs=2) as slp, \
                 tc.tile_pool(name="proto", bufs=CB) as prp, \
                 tc.tile_pool(name="psq", bufs=2) as psqp, \
                 tc.tile_pool(name="prps", bufs=1, space="PSUM") as prpsp, \
                 tc.tile_pool(name="pmisc", bufs=2) as miscp, \
                 tc.tile_pool(name="pbc", bufs=1) as pbcp:
                pn_ps = prpsp.tile([1, M], F32)
                protos = []
                for cb in range(CB):
                    sh_t = []
                    for sh in range(5):
                        t = slp.tile([128, M], F32, tag=f"sl{sh}")
                        nc.sync.dma_start(t[:], s_d[sh, cb * 128:(cb + 1) * 128, :])
                        sh_t.append(t)
                    p01 = miscp.tile([128, M], F32, tag="p01")
                    nc.vector.tensor_add(p01[:], sh_t[0][:], sh_t[1][:])
                    p23 = miscp.tile([128, M], F32, tag="p23")
                    nc.vector.tensor_add(p23[:], sh_t[2][:], sh_t[3][:])
                    p03 = miscp.tile([128, M], F32, tag="p03")
                    nc.vector.tensor_add(p03[:], p01[:], p23[:])
                    pr = prp.tile([128, M], F32, tag="proto")
                    nc.vector.tensor_add(pr[:], p03[:], sh_t[4][:])
                    protos.append(pr)
                    sq = psqp.tile([128, M], F32R, tag="psq")
                    nc.scalar.square(sq[:], pr[:])
                    for h in range(2):
                        nc.tensor.matmul(pn_ps[:, h * 512:(h + 1) * 512], ones_r[:],
                                         sq[:, h * 512:(h + 1) * 512],
                                         start=(cb == 0), stop=(cb == CB - 1))
                pnorm = miscp.tile([1, M], F32, tag="pn")
                nc.scalar.sqrt(pnorm[:], pn_ps[:])
                pinv = miscp.tile([1, M], F32, tag="pinv")
                nc.vector.reciprocal(pinv[:], pnorm[:])
                pbc = pbcp.tile([128, M], F32, tag="pbc")
                nc.gpsimd.partition_broadcast(pbc[:], pinv[:])
                for cb in range(CB):
                    sht = shat_p.tile([128, M], F32R, tag="shat")
                    nc.vector.tensor_mul(sht[:], protos[cb][:], pbc[:])
                    shat.append(sht)

            # ---------------- main loop over query images ----------------
            with tc.tile_pool(name="qld", bufs=2) as qlp, \
                 tc.tile_pool(name="qsq", bufs=2) as qsqp, \
                 tc.tile_pool(name="simps", bufs=2, space="PSUM") as simp, \
                 tc.tile_pool(name="qnps", bufs=2, space="PSUM") as qnp, \
                 tc.tile_pool(name="stash", bufs=2) as stp:
                for img in range(NQ_SH):
                    qts = []
                    for cb in range(CB):
                        t = qlp.tile([128, N], F32R, tag=f"q{cb}")
                        nc.sync.dma_start(
                            t[:], q_d[img, cb * 128:(cb + 1) * 128, :].bitcast(F32R))
                        qts.append(t)
                    # query-patch squared norms: ones-matmul column sums
                    qn_ps = qnp.tile([1, N], F32)
                    for cb in range(CB):
                        sq = qsqp.tile([128, N], F32R, tag="qsq")
                        nc.scalar.square(sq[:], qts[cb][:])
                        for h in range(2):
                            nc.tensor.matmul(qn_ps[:, h * 512:(h + 1) * 512],
                                             ones_r[:], sq[:, h * 512:(h + 1) * 512],
                                             start=(cb == 0), stop=(cb == CB - 1))
                    st = stp.tile([1, N], F32, tag="stash")
                    nc.scalar.copy(st[:], qn_ps[:])
                    nc.sync.dma_start(norms2_all[img:img + 1, :], st[:])
                    # similarity tiles + row max
                    for nb in range(8):
                        ps = simp.tile([128, M], F32)
                        for cb in range(CB):
                            lhsT = qts[cb][:, nb * 128:(nb + 1) * 128]
                            for h in range(2):
                                nc.tensor.matmul(ps[:, h * 512:(h + 1) * 512], lhsT,
                                                 shat[cb][:, h * 512:(h + 1) * 512],
                                                 start=(cb == 0), stop=(cb == CB - 1))
                        col = nb * 16 + img
                        nc.vector.reduce_max(maxcols[:, col:col + 1], ps[:, :],
                                             axis=AX_X)

            # ---------------- epilogue: scores ----------------
            with tc.tile_pool(name="ep", bufs=1) as ep, \
                 tc.tile_pool(name="eps", bufs=4, space="PSUM") as epp:
                qn = ep.tile([NQ_SH, N], F32, tag="qn")
                nc.scalar.sqrt(qn[:], norms2_all[:])
                qinv = ep.tile([NQ_SH, N], F32, tag="qinv")
                nc.vector.reciprocal(qinv[:], qn[:])
                scaled = ep.tile([128, 128], F32, tag="scaled")
                for nb in range(8):
                    tr = epp.tile([128, 16], F32, tag="tr")
                    nc.tensor.transpose(tr[:], qinv[:, nb * 128:(nb + 1) * 128],
                                        ident16[:])
                    nc.vector.scalar_tensor_tensor(
                        out=scaled[:, nb * 16:(nb + 1) * 16],
                        in0=maxcols[:, nb * 16:(nb + 1) * 16],
                        scalar=1.0 / float(N),
                        in1=tr[:], op0=MULT, op1=MULT)
                fin_ps = epp.tile([1, 128], F32, tag="fin")
                nc.tensor.matmul(fin_ps[:], ones_f[:], scaled[:],
                                 start=True, stop=True)
                fin_sb = ep.tile([1, 128], F32, tag="finsb")
                nc.scalar.copy(fin_sb[:], fin_ps[:])
                scores = ep.tile([1, NQ_SH], F32, tag="scores")
                nc.vector.reduce_sum(
                    scores[:],
                    fin_sb[:].rearrange("p (b i) -> p i b", b=8, i=16),
                    axis=AX_X)
                nc.sync.dma_start(out_d[:, :], scores[:])

    nc.compile()
    return nc


_NC_CACHE = None


def _get_nc():
    global _NC_CACHE
    if _NC_CACHE is None:
        _NC_CACHE = build_bass()
    return _NC_CACHE


def kernel(query_features: np.ndarray, support_features: np.ndarray) -> np.ndarray:
    NQ = query_features.shape[0]
    assert NQ == NQ_SH * NCORES
    q = np.ascontiguousarray(
        query_features.reshape(NQ, CB * 128, N).astype(np.float32, copy=False))
    s = np.ascontiguousarray(
        support_features.reshape(5, CB * 128, M).astype(np.float32, copy=False))
    nc = _get_nc()
    in_maps = [
        {"q": q[i * NQ_SH:(i + 1) * NQ_SH], "s": s} for i in range(NCORES)
    ]
    res = bass_utils.run_bass_kernel_spmd(nc, in_maps, core_ids=list(range(NCORES)))
    out = np.concatenate(
        [np.asarray(res.results[i]["out"]).reshape(NQ_SH) for i in range(NCORES)])
    return out.astype(np.float32, copy=False)
